# revision 43
# baseline (speedup 1.0000x reference)
"""Expert-parallel MoE SwiGLU kernel for 8 Trainium2 NeuronCores.

Problem: N=4096 tokens, top-2 of E=8 experts, H=2048, I=1408, fp32.

Strategy (expert parallel, per the sharding hint):
  - Host-side dispatch: gather each expert's routed tokens (the "all-to-all
    dispatch" step) while sharding the full inputs; core e gets expert e's
    token slab x_e^T [H, C] plus its weight triple (transposed).
  - Device: each core independently computes
        y_e^T = Wd_e @ (silu(Wg_e @ x_e^T) * (Wu_e @ x_e^T))
    entirely in [feature, token] layout so no on-device transposes are
    needed. Matmuls run in bf16; output returns in bf16 (the final
    rounding adds ~2e-3 absmax-rel on top of the ~4e-3 bf16 compute).
  - Host-side combine: weighted scatter-add of per-expert outputs back to
    the [N, H] output (the "all-to-all combine" step).

Capacity C = max tokens routed to one expert padded to a multiple of 4
(bf16 matmuls have no minimum moving-dim; no 128 padding needed), so the
PE stream is 528*C cycles.  DMA count is kept low (pair-merged weight /
x / output transfers) because every allocated semaphore costs ~4x25ns in
the serialized end-of-kernel reset ceremony, which sits on the critical
path once the output drain is fast.

Front choreography: phase-1 i=0 interleaves gate/up per h-chunk and the
front DMAs are issued in consumption order across both HWDGE rings
(sync + scalar), with the i=0 weight back halves and all later weights
on the gpsimd SWDGE ring.  i>=1 runs gate-then-up so the SWDGE prefetch
of wu keeps half a pass of slack.  Phase 2 reuses phase 1's PSUM banks
via tile tags (no pool-transition barrier) and alternates output DMAs
between the two rings, switching from row-pair DMAs to single-row DMAs
for the last rows so the final drain is short.
"""

import numpy as np

import concourse.bass as bass
import concourse.tile as tile
from concourse import bacc, mybir
from concourse import bass_utils

N, K, E, H, I = 4096, 2, 8, 2048, 1408
P = 128
HCH = H // P   # 16 chunks over hidden dim
ICH = I // P   # 11 chunks over intermediate dim
F32 = mybir.dt.float32
F32R = mybir.dt.float32r
BF16 = mybir.dt.bfloat16


def _chunks(C):
    """Split C (multiple of 4) into near-equal col chunks <= 512, 4-aligned.
    Measured: chunk-group pacing is PE-stream + ~3.6 ns per matmul (NX
    dispatch + semaphore + LDW shadow), invariant to chunk widths/alignment
    — (512,512,108) and (380,376,376) pace identically — so minimal C wins
    and the split shape is free."""
    assert C % 4 == 0
    n = max(1, -(-C // 512))
    base = C // n // 4 * 4
    sizes = [base] * n
    rem = C - base * n
    i = 0
    while rem > 0:
        sizes[i] += 4
        rem -= 4
        i = (i + 1) % n
    out, off = [], 0
    for s in sizes:
        out.append((off, s))
        off += s
    assert off == C and all(s <= 512 for _, s in out)
    return out


def _build(C, xdt=BF16, wdt=BF16, hdt=BF16):
    """Build + compile the per-core SwiGLU kernel for capacity C."""
    ch = _chunks(C)
    nc = bacc.Bacc("TRN2", target_bir_lowering=False, debug=False,
                   enable_asserts=False, num_devices=E)

    xT = nc.dram_tensor("xT", [H, C], xdt, kind="ExternalInput")
    # weights come host-pre-tiled so every DMA line is contiguous:
    # wgp[i, p, h*128+j] = Wg[e][i*128+j, h*128+p]  (lhsT tiles back to back)
    wgp = nc.dram_tensor("wgp", [ICH, P, H], wdt, kind="ExternalInput")
    wup = nc.dram_tensor("wup", [ICH, P, H], wdt, kind="ExternalInput")
    wdp = nc.dram_tensor("wdp", [HCH, P, I], wdt, kind="ExternalInput")
    outT = nc.dram_tensor("outT", [H, C], BF16, kind="ExternalOutput")

    x_r = xT.ap().rearrange("(ho p) c -> p ho c", p=P)      # [128, 16, C]
    wg_r = wgp.ap()
    wu_r = wup.ap()
    wg_p = wgp.ap().rearrange("i p h -> p i h")             # [128, 11, H]
    wu_p = wup.ap().rearrange("i p h -> p i h")
    wd_p = wdp.ap().rearrange("h p i -> p h i")             # [128, 16, I]
    out_r = outT.ap().rearrange("(ho p) c -> p ho c", p=P)  # [128, 16, C]

    with tile.TileContext(nc) as tc:
        with (
            tc.tile_pool(name="xpool", bufs=1) as xpool,
            tc.tile_pool(name="hpool", bufs=1) as hpool,
            tc.tile_pool(name="w0pool", bufs=1) as w0pool,
            tc.tile_pool(name="wpool", bufs=2) as wpool,
            tc.tile_pool(name="dpool", bufs=2) as dpool,
            tc.tile_pool(name="opool", bufs=2) as opool,
        ):
            # resident activations: x^T and hidden^T
            x_sb = xpool.tile([P, HCH, C], xdt)
            hid_sb = hpool.tile([P, ICH, C], hdt)
            w_sb0 = w0pool.tile([P, 2, H], wdt, name="w_sb_0")
            # fixed 384-col warmup shape, independent of the chunk split,
            # so the warmup bridge duration stays tuned to the x0 arrival
            # band; uses the spare 7th PSUM bank
            warm = w0pool.tile([P, 384], wdt, name="warm")
            from concourse.tile import add_dep_helper

            # Front choreography.  i=0 consumes (wg[h], wu[h], x[h]) in h
            # order at ~0.95us per h.  Two constraints shape this: the DMA
            # engines start cold (big 2KB+ lines only), and walrus rotates
            # only ~5 completion semaphores per hardware queue — a 6th DMA
            # issue on a ring stalls until the 1st fully completes.  So each
            # ring gets at most ~6 front DMAs, consecutive rows paired.
            # The i=0 weight back halves (needed only ~7.6us after PE
            # start) and all later weights ride SWDGE, dep-gated so they
            # don't steal front HBM bandwidth from the x stream.
            #   sync  : wg0[h0:8], x1, x3, x5, x7, x10+11, x14+15
            #   scalar: x0, wu0[h0:8], x2, x4, x6, x8+9, x12+13
            x_dma = {}
            nc.sync.dma_start(w_sb0[:, 0, 0:8 * P], wg_r[0][:, 0:8 * P])
            x_dma[0] = nc.scalar.dma_start(x_sb[:, 0, :], x_r[:, 0, :])
            nc.scalar.dma_start(w_sb0[:, 1, 0:8 * P], wu_r[0][:, 0:8 * P])
            nc.sync.dma_start(x_sb[:, 1, :], x_r[:, 1, :])
            nc.scalar.dma_start(x_sb[:, 2, :], x_r[:, 2, :])
            nc.sync.dma_start(x_sb[:, 3, :], x_r[:, 3, :])
            nc.scalar.dma_start(x_sb[:, 4, :], x_r[:, 4, :])
            nc.sync.dma_start(x_sb[:, 5, :], x_r[:, 5, :])
            nc.scalar.dma_start(x_sb[:, 6, :], x_r[:, 6, :])
            nc.sync.dma_start(x_sb[:, 7, :], x_r[:, 7, :])
            x_dma[8] = nc.scalar.dma_start(x_sb[:, 8:10, :], x_r[:, 8:10, :])
            nc.sync.dma_start(x_sb[:, 10:12, :], x_r[:, 10:12, :])
            x_dma[12] = nc.scalar.dma_start(x_sb[:, 12:14, :], x_r[:, 12:14, :])
            x_dma[14] = nc.sync.dma_start(x_sb[:, 14:16, :], x_r[:, 14:16, :])
            # hold the i=0 back-half weights until x0 is in; they are not
            # consumed until ~7.6us after PE start, and issuing them cold
            # steals DMA-engine/HBM bandwidth from the first x0/wg pieces
            # that gate the first matmul
            dh0 = nc.gpsimd.dma_start(w_sb0[:, 0, 8 * P:], wg_r[0][:, 8 * P:])
            add_dep_helper(dh0.ins, x_dma[0].ins, reason="yield front BW to x0")
            dh1 = nc.gpsimd.dma_start(w_sb0[:, 1, 8 * P:], wu_r[0][:, 8 * P:])
            add_dep_helper(dh1.ins, x_dma[0].ins, reason="yield front BW to x0")

            # one PSUM pool for both phases: phase 2 reuses phase 1's six
            # banks via tags, so there is no pool-transition barrier
            with tc.tile_pool(name="ps", bufs=1, space="PSUM") as psp:
                # PE warmup: the HAM clock gate starts the tensor engine at
                # reduced clock and only reaches 8/8 after ~3us of sustained
                # activity.  The front is DMA-bound anyway, so burn the wait
                # on dummy matmuls over a memset tile; the real matmuls then
                # start at full clock.
                nc.vector.memset(warm[:], 0)
                # bridge length targets the HIGH edge of the x0-arrival
                # band (~12.9us): undershooting idles the PE into a clock
                # dip costing ~2x the idle, overshooting costs only 1x
                ps_w = psp.tile([P, 384], F32, name="ps_warm", tag="warm")
                for _ in range(22):
                    nc.tensor.matmul(ps_w[:], warm[:, 0:P], warm[:],
                                     start=True, stop=True)
                # ---- phase 1: gate/up projections + SwiGLU -> hidden^T
                for i in range(ICH):
                    if i == 0:
                        w_sb = w_sb0
                    elif i % 2 == 1:
                        # i-pair prefetch: one DMA per weight tensor covers
                        # (i, i+1); half-pass slack before wu is consumed
                        wpair = wpool.tile([P, 2, 2, H], wdt, tag="w12",
                                           name=f"w_sb_{i}")
                        ilim = min(2, ICH - i)
                        d0 = nc.gpsimd.dma_start(wpair[:, 0:ilim, 0, :],
                                                 wg_p[:, i:i + ilim, :])
                        d1 = nc.gpsimd.dma_start(wpair[:, 0:ilim, 1, :],
                                                 wu_p[:, i:i + ilim, :])
                        if i == 1:
                            # hold the first prefetch until x is mostly in
                            # flight so the front HBM bandwidth goes to x
                            # and the i=0 back-half weights on this ring
                            add_dep_helper(d0.ins, x_dma[8].ins,
                                           reason="yield front BW to x")
                            add_dep_helper(d1.ins, x_dma[12].ins,
                                           reason="yield front BW to x")
                    ps_g = [
                        psp.tile([P, cw], F32, name=f"psg_{i}_{n}", tag=f"psg{n}")
                        for n, (c0, cw) in enumerate(ch)
                    ]
                    ps_u = [
                        psp.tile([P, cw], F32, name=f"psu_{i}_{n}", tag=f"psu{n}")
                        for n, (c0, cw) in enumerate(ch)
                    ]
                    # i=0: h outer, gate/up inner so the PE consumes x[h]
                    # at the paced rate the front choreography delivers it.
                    # i>=1: gate/up outer so wu keeps prefetch slack.
                    if i == 0:
                        loop = [(h, m) for h in range(HCH) for m in (0, 1)]
                    else:
                        loop = [(h, m) for m in (0, 1) for h in range(HCH)]
                    for h, m in loop:
                        ps = ps_g if m == 0 else ps_u
                        if i == 0:
                            lhsT = w_sb0[:, m, h * P:(h + 1) * P]
                        else:
                            lhsT = wpair[:, (i - 1) % 2, m, h * P:(h + 1) * P]
                        for n, (c0, cw) in enumerate(ch):
                            nc.tensor.matmul(
                                ps[n][:],
                                lhsT,
                                x_sb[:, h, c0:c0 + cw],
                                start=(h == 0),
                                stop=(h == HCH - 1),
                            )
                    for n, (c0, cw) in enumerate(ch):
                        hs = hid_sb[:, i, c0:c0 + cw]
                        nc.scalar.activation(
                            out=hs, in_=ps_g[n][:],
                            func=mybir.ActivationFunctionType.Silu,
                        )
                        nc.vector.tensor_mul(out=hs, in0=hs, in1=ps_u[n][:])

                # ---- phase 2: down projection -> out^T [H, C]
                o_pair = None
                for h in range(HCH):
                    if h % 4 == 0:
                        # 4 h-tiles of down weights per DMA
                        wd_sb = dpool.tile([P, 4, I], wdt, tag="wd")
                        dd = nc.gpsimd.dma_start(wd_sb[:],
                                                 wd_p[:, h:h + 4, :])
                        if h == 0:
                            add_dep_helper(dd.ins, x_dma[14].ins,
                                           reason="yield front BW to x")
                    # phase 2 reuses phase 1's PSUM banks, double-buffered
                    # by alternating between the psg and psu tag sets
                    tg = "psg" if h % 2 == 0 else "psu"
                    ps_d = [
                        psp.tile([P, cw], F32, name=f"psd_{h}_{n}", tag=f"{tg}{n}")
                        for n, (c0, cw) in enumerate(ch)
                    ]
                    for i in range(ICH):
                        lhsT = wd_sb[:, h % 4, i * P:(i + 1) * P]
                        for n, (c0, cw) in enumerate(ch):
                            nc.tensor.matmul(
                                ps_d[n][:],
                                lhsT,
                                hid_sb[:, i, c0:c0 + cw],
                                start=(i == 0),
                                stop=(i == ICH - 1),
                            )
                    if h % 2 == 0:
                        o_pair = opool.tile([P, 2, C], BF16, tag="o")
                    # row-pair output DMAs mid-phase (fewer issues), then
                    # finer pieces toward the end so the final drain after
                    # the last matmul is short; rings alternate throughout
                    if h < 12:
                        for n, (c0, cw) in enumerate(ch):
                            nc.vector.tensor_copy(o_pair[:, h % 2, c0:c0 + cw],
                                                  ps_d[n][:])
                        if h % 2 == 1:
                            oeng = nc.sync if h % 4 == 1 else nc.scalar
                            oeng.dma_start(out_r[:, h - 1:h + 1, :], o_pair[:])
                    elif h < 14:
                        for n, (c0, cw) in enumerate(ch):
                            nc.vector.tensor_copy(o_pair[:, h % 2, c0:c0 + cw],
                                                  ps_d[n][:])
                        oeng = nc.sync if h % 2 == 0 else nc.scalar
                        oeng.dma_start(out_r[:, h, :], o_pair[:, h % 2, :])
                    else:
                        # last two rows: per-col-chunk casts alternating
                        # between DVE and the idle ACT engine, each chunk
                        # DMA'd immediately with rings alternating, so the
                        # post-last-matmul drain is short (~2.7us)
                        for n, (c0, cw) in enumerate(ch):
                            dst = o_pair[:, h % 2, c0:c0 + cw]
                            if n % 2 == 0:
                                nc.vector.tensor_copy(dst, ps_d[n][:])
                            else:
                                nc.scalar.activation(
                                    out=dst, in_=ps_d[n][:],
                                    func=mybir.ActivationFunctionType.Copy,
                                )
                            oeng = nc.sync if (h + n) % 2 == 0 else nc.scalar
                            oeng.dma_start(out_r[:, h, c0:c0 + cw], dst)

    nc.compile()
    return nc


_NC_CACHE = {}

# compute dtype config: "bf16" (default) or "f32r" (FP22 single-pass)
DTYPES = {
    "f32r": (F32R, F32R, F32R),
    "bf16": (BF16, BF16, BF16),
    "xbf16": (BF16, F32R, F32R),
}
import os
CONFIG = os.environ.get("MOE_KERNEL_CONFIG", "bf16")


def _get_nc(C):
    key = (C, CONFIG)
    if key not in _NC_CACHE:
        _NC_CACHE[key] = _build(C, *DTYPES[CONFIG])
    return _NC_CACHE[key]


def kernel(x, topk_ids, topk_weight, Wg, Wu, Wd):
    x = np.asarray(x, dtype=np.float32)
    topk_ids = np.asarray(topk_ids)
    topk_weight = np.asarray(topk_weight, dtype=np.float32)

    # ---- host-side dispatch (the all-to-all by topk_ids)
    flat = topk_ids.reshape(-1).astype(np.int64)
    order = np.argsort(flat, kind="stable")
    counts = np.bincount(flat, minlength=E)
    toks = order // K          # token index per sorted slot
    ks = order % K             # which of the top-k slots
    bounds = np.cumsum(counts)
    starts = bounds - counts

    C = max(16, int(-(-counts.max() // 4)) * 4)
    nc = _get_nc(C)

    import ml_dtypes
    xdt, wdt, _ = DTYPES[CONFIG]
    np_x = ml_dtypes.bfloat16 if xdt == BF16 else np.float32
    np_w = ml_dtypes.bfloat16 if wdt == BF16 else np.float32

    def pack_gu(w):  # [I, H] -> [ICH, P, H]; out[i, p, h*128+j] = w[i*128+j, h*128+p]
        v = np.asarray(w, np.float32).reshape(ICH, P, HCH, P)       # [i, j, h, p]
        return np.ascontiguousarray(
            v.transpose(0, 3, 2, 1).astype(np_w)).reshape(ICH, P, H)

    def pack_d(w):   # [H, I] -> [HCH, P, I]; out[h, p, i*128+j] = w[h*128+j, i*128+p]
        v = np.asarray(w, np.float32).reshape(HCH, P, ICH, P)       # [h, j, i, p]
        return np.ascontiguousarray(
            v.transpose(0, 3, 2, 1).astype(np_w)).reshape(HCH, P, I)

    in_maps = []
    tok_e, k_e = [], []
    for e in range(E):
        te = toks[starts[e]:bounds[e]]
        ke = ks[starts[e]:bounds[e]]
        tok_e.append(te)
        k_e.append(ke)
        xT_e = np.zeros((H, C), np_x)
        xT_e[:, :len(te)] = x[te].T.astype(np_x)
        in_maps.append({
            "xT": xT_e,
            "wgp": pack_gu(Wg[e]),
            "wup": pack_gu(Wu[e]),
            "wdp": pack_d(Wd[e]),
        })

    res = bass_utils.run_bass_kernel_spmd(nc, in_maps, core_ids=list(range(E)))

    # ---- host-side combine (weighted scatter-add)
    out = np.zeros((N, H), np.float32)
    for e in range(E):
        te, ke = tok_e[e], k_e[e]
        if len(te) == 0:
            continue
        yT = np.asarray(res.results[e]["outT"][:, :len(te)],
                        dtype=np.float32)                 # [H, count]
        w = topk_weight[te, ke].astype(np.float32)
        out[te] += (yT * w[None, :]).T
    return out


# revision 44
# speedup vs baseline: 1.0050x; 1.0050x over previous
"""Expert-parallel MoE SwiGLU kernel for 8 Trainium2 NeuronCores.

Problem: N=4096 tokens, top-2 of E=8 experts, H=2048, I=1408, fp32.

Strategy (expert parallel, per the sharding hint):
  - Host-side dispatch: gather each expert's routed tokens (the "all-to-all
    dispatch" step) while sharding the full inputs; core e gets expert e's
    token slab x_e^T [H, C] plus its weight triple (transposed).
  - Device: each core independently computes
        y_e^T = Wd_e @ (silu(Wg_e @ x_e^T) * (Wu_e @ x_e^T))
    entirely in [feature, token] layout so no on-device transposes are
    needed. Matmuls run in bf16; output returns in bf16 (the final
    rounding adds ~2e-3 absmax-rel on top of the ~4e-3 bf16 compute).
  - Host-side combine: weighted scatter-add of per-expert outputs back to
    the [N, H] output (the "all-to-all combine" step).

Capacity C = max tokens routed to one expert padded to a multiple of 4
(bf16 matmuls have no minimum moving-dim; no 128 padding needed), so the
PE stream is 528*C cycles.  DMA count is kept low (pair-merged weight /
x / output transfers) because every allocated semaphore costs ~4x25ns in
the serialized end-of-kernel reset ceremony, which sits on the critical
path once the output drain is fast.

Front choreography: phase-1 i=0 interleaves gate/up per h-chunk and the
front DMAs are issued in consumption order across both HWDGE rings
(sync + scalar), with the i=0 weight back halves and all later weights
on the gpsimd SWDGE ring.  i>=1 runs gate-then-up so the SWDGE prefetch
of wu keeps half a pass of slack.  Phase 2 reuses phase 1's PSUM banks
via tile tags (no pool-transition barrier) and alternates output DMAs
between the two rings, switching from row-pair DMAs to single-row DMAs
for the last rows so the final drain is short.
"""

import numpy as np

import concourse.bass as bass
import concourse.tile as tile
from concourse import bacc, mybir
from concourse import bass_utils

N, K, E, H, I = 4096, 2, 8, 2048, 1408
P = 128
HCH = H // P   # 16 chunks over hidden dim
ICH = I // P   # 11 chunks over intermediate dim
F32 = mybir.dt.float32
F32R = mybir.dt.float32r
BF16 = mybir.dt.bfloat16


def _chunks(C):
    """Split C (multiple of 4) into near-equal col chunks <= 512, 4-aligned.
    Measured: chunk-group pacing is PE-stream + ~3.6 ns per matmul (NX
    dispatch + semaphore + LDW shadow), invariant to chunk widths/alignment
    — (512,512,108) and (380,376,376) pace identically — so minimal C wins
    and the split shape is free."""
    assert C % 4 == 0
    n = max(1, -(-C // 512))
    base = C // n // 4 * 4
    sizes = [base] * n
    rem = C - base * n
    i = 0
    while rem > 0:
        sizes[i] += 4
        rem -= 4
        i = (i + 1) % n
    out, off = [], 0
    for s in sizes:
        out.append((off, s))
        off += s
    assert off == C and all(s <= 512 for _, s in out)
    return out


def _build(C, xdt=BF16, wdt=BF16, hdt=BF16):
    """Build + compile the per-core SwiGLU kernel for capacity C."""
    ch = _chunks(C)
    nc = bacc.Bacc("TRN2", target_bir_lowering=False, debug=False,
                   enable_asserts=False, num_devices=E)

    xT = nc.dram_tensor("xT", [H, C], xdt, kind="ExternalInput")
    # weights come host-pre-tiled so every DMA line is contiguous:
    # wgp[i, p, h*128+j] = Wg[e][i*128+j, h*128+p]  (lhsT tiles back to back)
    wgp = nc.dram_tensor("wgp", [ICH, P, H], wdt, kind="ExternalInput")
    wup = nc.dram_tensor("wup", [ICH, P, H], wdt, kind="ExternalInput")
    wdp = nc.dram_tensor("wdp", [HCH, P, I], wdt, kind="ExternalInput")
    outT = nc.dram_tensor("outT", [H, C], BF16, kind="ExternalOutput")

    x_r = xT.ap().rearrange("(ho p) c -> p ho c", p=P)      # [128, 16, C]
    wg_r = wgp.ap()
    wu_r = wup.ap()
    wg_p = wgp.ap().rearrange("i p h -> p i h")             # [128, 11, H]
    wu_p = wup.ap().rearrange("i p h -> p i h")
    wd_p = wdp.ap().rearrange("h p i -> p h i")             # [128, 16, I]
    out_r = outT.ap().rearrange("(ho p) c -> p ho c", p=P)  # [128, 16, C]

    with tile.TileContext(nc) as tc:
        with (
            tc.tile_pool(name="xpool", bufs=1) as xpool,
            tc.tile_pool(name="hpool", bufs=1) as hpool,
            tc.tile_pool(name="w0pool", bufs=1) as w0pool,
            tc.tile_pool(name="wpool", bufs=2) as wpool,
            tc.tile_pool(name="dpool", bufs=2) as dpool,
            tc.tile_pool(name="opool", bufs=2) as opool,
        ):
            # resident activations: x^T and hidden^T
            x_sb = xpool.tile([P, HCH, C], xdt)
            hid_sb = hpool.tile([P, ICH, C], hdt)
            w_sb0 = w0pool.tile([P, 2, H], wdt, name="w_sb_0")
            # fixed 384-col warmup shape, independent of the chunk split,
            # so the warmup bridge duration stays tuned to the x0 arrival
            # band; uses the spare 7th PSUM bank
            warm = w0pool.tile([P, 384], wdt, name="warm")
            from concourse.tile import add_dep_helper

            # Front choreography.  i=0 consumes (wg[h], wu[h], x[h]) in h
            # order at ~0.95us per h.  Two constraints shape this: the DMA
            # engines start cold (big 2KB+ lines only), and walrus rotates
            # only ~5 completion semaphores per hardware queue — a 6th DMA
            # issue on a ring stalls until the 1st fully completes.  So each
            # ring gets at most ~6 front DMAs, consecutive rows paired.
            # The i=0 weight back halves (needed only ~7.6us after PE
            # start) and all later weights ride SWDGE, dep-gated so they
            # don't steal front HBM bandwidth from the x stream.
            #   sync  : wg0[h0:8], x1, x3, x5, x7, x10+11, x14+15
            #   scalar: x0, wu0[h0:8], x2, x4, x6, x8+9, x12+13
            x_dma = {}
            nc.sync.dma_start(w_sb0[:, 0, 0:8 * P], wg_r[0][:, 0:8 * P])
            x_dma[0] = nc.scalar.dma_start(x_sb[:, 0, :], x_r[:, 0, :])
            nc.scalar.dma_start(w_sb0[:, 1, 0:8 * P], wu_r[0][:, 0:8 * P])
            nc.sync.dma_start(x_sb[:, 1, :], x_r[:, 1, :])
            nc.scalar.dma_start(x_sb[:, 2, :], x_r[:, 2, :])
            nc.sync.dma_start(x_sb[:, 3, :], x_r[:, 3, :])
            nc.scalar.dma_start(x_sb[:, 4, :], x_r[:, 4, :])
            nc.sync.dma_start(x_sb[:, 5, :], x_r[:, 5, :])
            nc.scalar.dma_start(x_sb[:, 6, :], x_r[:, 6, :])
            nc.sync.dma_start(x_sb[:, 7, :], x_r[:, 7, :])
            x_dma[8] = nc.scalar.dma_start(x_sb[:, 8:10, :], x_r[:, 8:10, :])
            nc.sync.dma_start(x_sb[:, 10:12, :], x_r[:, 10:12, :])
            x_dma[12] = nc.scalar.dma_start(x_sb[:, 12:14, :], x_r[:, 12:14, :])
            x_dma[14] = nc.sync.dma_start(x_sb[:, 14:16, :], x_r[:, 14:16, :])
            # hold the i=0 back-half weights until x0 is in; they are not
            # consumed until ~7.6us after PE start, and issuing them cold
            # steals DMA-engine/HBM bandwidth from the first x0/wg pieces
            # that gate the first matmul
            dh0 = nc.gpsimd.dma_start(w_sb0[:, 0, 8 * P:], wg_r[0][:, 8 * P:])
            add_dep_helper(dh0.ins, x_dma[0].ins, reason="yield front BW to x0")
            dh1 = nc.gpsimd.dma_start(w_sb0[:, 1, 8 * P:], wu_r[0][:, 8 * P:])
            add_dep_helper(dh1.ins, x_dma[0].ins, reason="yield front BW to x0")

            # one PSUM pool for both phases: phase 2 reuses phase 1's six
            # banks via tags, so there is no pool-transition barrier
            with tc.tile_pool(name="ps", bufs=1, space="PSUM") as psp:
                # PE warmup: the HAM clock gate starts the tensor engine at
                # reduced clock and only reaches 8/8 after ~3us of sustained
                # activity.  The front is DMA-bound anyway, so burn the wait
                # on dummy matmuls over a memset tile; the real matmuls then
                # start at full clock.
                nc.vector.memset(warm[:], 0)
                ps_w = psp.tile([P, 384], F32, name="ps_warm", tag="warm")
                for _ in range(20):
                    nc.tensor.matmul(ps_w[:], warm[:, 0:P], warm[:],
                                     start=True, stop=True)
                # ---- phase 1: gate/up projections + SwiGLU -> hidden^T
                for i in range(ICH):
                    if i == 0:
                        w_sb = w_sb0
                    elif i % 2 == 1:
                        # i-pair prefetch: one DMA per weight tensor covers
                        # (i, i+1); half-pass slack before wu is consumed
                        wpair = wpool.tile([P, 2, 2, H], wdt, tag="w12",
                                           name=f"w_sb_{i}")
                        ilim = min(2, ICH - i)
                        d0 = nc.gpsimd.dma_start(wpair[:, 0:ilim, 0, :],
                                                 wg_p[:, i:i + ilim, :])
                        d1 = nc.gpsimd.dma_start(wpair[:, 0:ilim, 1, :],
                                                 wu_p[:, i:i + ilim, :])
                        if i == 1:
                            # hold the first prefetch until x is mostly in
                            # flight so the front HBM bandwidth goes to x
                            # and the i=0 back-half weights on this ring
                            add_dep_helper(d0.ins, x_dma[8].ins,
                                           reason="yield front BW to x")
                            add_dep_helper(d1.ins, x_dma[12].ins,
                                           reason="yield front BW to x")
                    ps_g = [
                        psp.tile([P, cw], F32, name=f"psg_{i}_{n}", tag=f"psg{n}")
                        for n, (c0, cw) in enumerate(ch)
                    ]
                    ps_u = [
                        psp.tile([P, cw], F32, name=f"psu_{i}_{n}", tag=f"psu{n}")
                        for n, (c0, cw) in enumerate(ch)
                    ]
                    # i=0: h outer, gate/up inner so the PE consumes x[h]
                    # at the paced rate the front choreography delivers it.
                    # i>=1: gate/up outer so wu keeps prefetch slack.
                    if i == 0:
                        loop = [(h, m) for h in range(HCH) for m in (0, 1)]
                    else:
                        loop = [(h, m) for m in (0, 1) for h in range(HCH)]
                    for h, m in loop:
                        ps = ps_g if m == 0 else ps_u
                        if i == 0:
                            lhsT = w_sb0[:, m, h * P:(h + 1) * P]
                        else:
                            lhsT = wpair[:, (i - 1) % 2, m, h * P:(h + 1) * P]
                        for n, (c0, cw) in enumerate(ch):
                            nc.tensor.matmul(
                                ps[n][:],
                                lhsT,
                                x_sb[:, h, c0:c0 + cw],
                                start=(h == 0),
                                stop=(h == HCH - 1),
                            )
                    for n, (c0, cw) in enumerate(ch):
                        hs = hid_sb[:, i, c0:c0 + cw]
                        nc.scalar.activation(
                            out=hs, in_=ps_g[n][:],
                            func=mybir.ActivationFunctionType.Silu,
                        )
                        nc.vector.tensor_mul(out=hs, in0=hs, in1=ps_u[n][:])

                # ---- phase 2: down projection -> out^T [H, C]
                o_pair = None
                for h in range(HCH):
                    if h % 4 == 0:
                        # 4 h-tiles of down weights per DMA
                        wd_sb = dpool.tile([P, 4, I], wdt, tag="wd")
                        dd = nc.gpsimd.dma_start(wd_sb[:],
                                                 wd_p[:, h:h + 4, :])
                        if h == 0:
                            add_dep_helper(dd.ins, x_dma[14].ins,
                                           reason="yield front BW to x")
                    # phase 2 reuses phase 1's PSUM banks, double-buffered
                    # by alternating between the psg and psu tag sets
                    tg = "psg" if h % 2 == 0 else "psu"
                    ps_d = [
                        psp.tile([P, cw], F32, name=f"psd_{h}_{n}", tag=f"{tg}{n}")
                        for n, (c0, cw) in enumerate(ch)
                    ]
                    for i in range(ICH):
                        lhsT = wd_sb[:, h % 4, i * P:(i + 1) * P]
                        for n, (c0, cw) in enumerate(ch):
                            nc.tensor.matmul(
                                ps_d[n][:],
                                lhsT,
                                hid_sb[:, i, c0:c0 + cw],
                                start=(i == 0),
                                stop=(i == ICH - 1),
                            )
                    if h % 2 == 0:
                        o_pair = opool.tile([P, 2, C], BF16, tag="o")
                    # row-pair output DMAs mid-phase (fewer issues), then
                    # finer pieces toward the end so the final drain after
                    # the last matmul is short; rings alternate throughout
                    if h < 12:
                        for n, (c0, cw) in enumerate(ch):
                            nc.vector.tensor_copy(o_pair[:, h % 2, c0:c0 + cw],
                                                  ps_d[n][:])
                        if h % 2 == 1:
                            oeng = nc.sync if h % 4 == 1 else nc.scalar
                            oeng.dma_start(out_r[:, h - 1:h + 1, :], o_pair[:])
                    elif h < 14:
                        for n, (c0, cw) in enumerate(ch):
                            nc.vector.tensor_copy(o_pair[:, h % 2, c0:c0 + cw],
                                                  ps_d[n][:])
                        oeng = nc.sync if h % 2 == 0 else nc.scalar
                        oeng.dma_start(out_r[:, h, :], o_pair[:, h % 2, :])
                    else:
                        # last two rows: per-col-chunk casts alternating
                        # between DVE and the idle ACT engine, each chunk
                        # DMA'd immediately with rings alternating, so the
                        # post-last-matmul drain is short (~2.7us)
                        for n, (c0, cw) in enumerate(ch):
                            dst = o_pair[:, h % 2, c0:c0 + cw]
                            if n % 2 == 0:
                                nc.vector.tensor_copy(dst, ps_d[n][:])
                            else:
                                nc.scalar.activation(
                                    out=dst, in_=ps_d[n][:],
                                    func=mybir.ActivationFunctionType.Copy,
                                )
                            oeng = nc.sync if (h + n) % 2 == 0 else nc.scalar
                            oeng.dma_start(out_r[:, h, c0:c0 + cw], dst)

    nc.compile()
    return nc


_NC_CACHE = {}

# compute dtype config: "bf16" (default) or "f32r" (FP22 single-pass)
DTYPES = {
    "f32r": (F32R, F32R, F32R),
    "bf16": (BF16, BF16, BF16),
    "xbf16": (BF16, F32R, F32R),
}
import os
CONFIG = os.environ.get("MOE_KERNEL_CONFIG", "bf16")


def _get_nc(C):
    key = (C, CONFIG)
    if key not in _NC_CACHE:
        _NC_CACHE[key] = _build(C, *DTYPES[CONFIG])
    return _NC_CACHE[key]


def kernel(x, topk_ids, topk_weight, Wg, Wu, Wd):
    x = np.asarray(x, dtype=np.float32)
    topk_ids = np.asarray(topk_ids)
    topk_weight = np.asarray(topk_weight, dtype=np.float32)

    # ---- host-side dispatch (the all-to-all by topk_ids)
    flat = topk_ids.reshape(-1).astype(np.int64)
    order = np.argsort(flat, kind="stable")
    counts = np.bincount(flat, minlength=E)
    toks = order // K          # token index per sorted slot
    ks = order % K             # which of the top-k slots
    bounds = np.cumsum(counts)
    starts = bounds - counts

    C = max(16, int(-(-counts.max() // 4)) * 4)
    nc = _get_nc(C)

    import ml_dtypes
    xdt, wdt, _ = DTYPES[CONFIG]
    np_x = ml_dtypes.bfloat16 if xdt == BF16 else np.float32
    np_w = ml_dtypes.bfloat16 if wdt == BF16 else np.float32

    def pack_gu(w):  # [I, H] -> [ICH, P, H]; out[i, p, h*128+j] = w[i*128+j, h*128+p]
        v = np.asarray(w, np.float32).reshape(ICH, P, HCH, P)       # [i, j, h, p]
        return np.ascontiguousarray(
            v.transpose(0, 3, 2, 1).astype(np_w)).reshape(ICH, P, H)

    def pack_d(w):   # [H, I] -> [HCH, P, I]; out[h, p, i*128+j] = w[h*128+j, i*128+p]
        v = np.asarray(w, np.float32).reshape(HCH, P, ICH, P)       # [h, j, i, p]
        return np.ascontiguousarray(
            v.transpose(0, 3, 2, 1).astype(np_w)).reshape(HCH, P, I)

    in_maps = []
    tok_e, k_e = [], []
    for e in range(E):
        te = toks[starts[e]:bounds[e]]
        ke = ks[starts[e]:bounds[e]]
        tok_e.append(te)
        k_e.append(ke)
        xT_e = np.zeros((H, C), np_x)
        xT_e[:, :len(te)] = x[te].T.astype(np_x)
        in_maps.append({
            "xT": xT_e,
            "wgp": pack_gu(Wg[e]),
            "wup": pack_gu(Wu[e]),
            "wdp": pack_d(Wd[e]),
        })

    res = bass_utils.run_bass_kernel_spmd(nc, in_maps, core_ids=list(range(E)))

    # ---- host-side combine (weighted scatter-add)
    out = np.zeros((N, H), np.float32)
    for e in range(E):
        te, ke = tok_e[e], k_e[e]
        if len(te) == 0:
            continue
        yT = np.asarray(res.results[e]["outT"][:, :len(te)],
                        dtype=np.float32)                 # [H, count]
        w = topk_weight[te, ke].astype(np.float32)
        out[te] += (yT * w[None, :]).T
    return out
